# revision 13
# baseline (speedup 1.0000x reference)
"""Trainium2 Bass kernel for nn_AttentionModule (segment_reduce).

Computation (per reference):
    wx   = features @ W
    s_g  = segment_sum(wx);  cnt_g = segment counts
    ctx  = tanh(s_g / max(cnt,1))            [G, D]
    score_n = <f_n, ctx[seg_n]>
    rep_g = segment_sum(score_n * f_n)       [G, D]

Key identity: segment_sum(F @ W) = segment_sum(F) @ W, so pass A is a pure
segment-sum.  Device algorithm (SPMD over 8 cores, segment-aligned shard
per core, one uniform compiled program; all data-dependence flows through
input tensors, never instruction addresses):

  per 4096-node chunk (32 new 128-node tiles + 1 boundary re-read tile):
    - pair one-hot oh01 [node, 32] host-built; segment-sum via PE matmuls
      into 32-aligned psum slot ranges; slot->window merge matmul (m01)
    - ctx = tanh((sum @ W) * recip)  [window 128, D]
    - unmerge matmul (m01T) redistributes ctx window rows back to pair
      slots: ctxp [slot 128, D]
    - transposed pair one-hot ohpT [strip*32+s, node] built on DVE via a
      single is_equal against a p%32 iota from a strip-packed broadcast
      relp4 (only 0.3MB/chunk vs 1.1MB for a full window one-hot)
    - ctx gather per tile: K=32 row-strip matmuls ohpT.T @ ctxp -> CtxG
      [node, D] in PSUM, evacuated to SBUF bf16 by ScalarE
    - scores = rowsum(F * CtxG): DVE 2x-mode bf16 multiply + single fused
      reduce;  rep via scores-scaled one-hot matmul (GpSimd does the
      scale), written chunk-major bf16 to DRAM
  host: numpy pre/post-processing (index metadata, shard assembly).
"""

import os
import sys
import math
from functools import lru_cache

for _p in ("/opt/trn_rl_repo", "/root/.axon_site/_ro/trn_rl_repo"):
    if os.path.isdir(_p) and _p not in sys.path:
        sys.path.insert(0, _p)

import numpy as np
import ml_dtypes

BF16 = ml_dtypes.bfloat16

P = 128          # partitions
TPC = 32         # new 128-node tiles per chunk
TILES = TPC + 1  # + boundary tile (tile 0)
WINW = 128       # chunk segment-window width
PAIRW = 32       # pair segment-window width
NCORES = 8
MASK = -1000.0

OHSC_ON_GPSIMD = bool(int(os.environ.get("K_OHSC_GPSIMD", "1")))
EVAC_DVE_GROUPS = int(os.environ.get("K_EVAC_DVE", "0"))  # of the 9 cg groups


def _strip_tiles(tpc):
    """tiles assigned to each of the 4 row strips, in block order."""
    strips = []
    for r in range(4):
        ts = []
        if r == 0:
            ts.append(0)
        for u in range(r, tpc // 2, 4):
            ts.extend([2 * u + 1, 2 * u + 2])
        strips.append(ts)
    return strips


def _ensure_ntff_hook():
    """Register the axon NTFF profile hook if the boot couldn't (antenv stub)."""
    import types
    try:
        import antenv  # noqa
    except ImportError:
        return
    if "antenv.axon_hooks" in sys.modules:
        return
    hooks = types.ModuleType("antenv.axon_hooks")
    holder = [None]
    hooks.set_axon_ntff_profile_hook = lambda h: holder.__setitem__(0, h)
    hooks.get_axon_ntff_profile_hook = lambda: holder[0]
    sys.modules["antenv.axon_hooks"] = hooks
    import antenv
    antenv.axon_hooks = hooks
    try:
        from trn_agent_boot.trn_boot import _ntff_profile_via_ctypes
        so = "/opt/axon/libaxon_pjrt.so"
        if os.path.exists(so):
            hooks.set_axon_ntff_profile_hook(_ntff_profile_via_ctypes(so))
    except Exception:
        pass


@lru_cache(maxsize=4)
def build_program(nch: int, tpc: int = TPC):
    """Build + compile the uniform per-core Bass program."""
    import concourse.bass as bass  # noqa
    import concourse.mybir as mybir
    from concourse import bacc, tile

    tiles = tpc + 1
    n_pairs = tpc // 2
    n_pt = (n_pairs + 3) // 4          # psum partial tiles (4 pairs each)
    n_g4 = (tiles + 3) // 4            # ctx-gather groups of 4 tiles
    swid = tiles * P                    # window one-hot width
    f32 = mybir.dt.float32
    bf16 = mybir.dt.bfloat16

    # byte offsets inside the packed meta tensor (bf16 elements)
    OH_OFF = 0
    OH_LEN = tiles * PAIRW
    M01_OFF = OH_OFF + OH_LEN
    M01_LEN = n_pt * WINW
    RC_OFF = M01_OFF + M01_LEN
    META_W = RC_OFF + 1

    nc = bacc.Bacc("TRN2", target_bir_lowering=False, debug=False,
                   num_devices=NCORES)

    # ---- DRAM I/O ----
    F_d = nc.dram_tensor("f_in", [nch, P, tiles, P], bf16,
                         kind="ExternalInput").ap()
    meta_d = nc.dram_tensor("meta", [nch, P, META_W], bf16,
                            kind="ExternalInput").ap()
    relp4_d = nc.dram_tensor("relp4", [nch, 1, swid], bf16,
                             kind="ExternalInput").ap()
    piota_d = nc.dram_tensor("piota", [P, 1], f32,
                             kind="ExternalInput").ap()
    W_d = nc.dram_tensor("w_in", [P, P], bf16, kind="ExternalInput").ap()
    rep_d = nc.dram_tensor("rep_out", [nch, P, n_pt, P], bf16,
                           kind="ExternalOutput").ap()

    AluOp = mybir.AluOpType
    Act = mybir.ActivationFunctionType

    pair_of = {t: ((t - 1) // 2 if t >= 1 else 0) for t in range(tiles)}

    with tile.TileContext(nc) as tc:
        with tc.tile_pool(name="const", bufs=1) as cpool, \
             tc.tile_pool(name="fpool", bufs=3) as fpool, \
             tc.tile_pool(name="mpool", bufs=3) as mpool, \
             tc.tile_pool(name="rpool", bufs=3) as rpool, \
             tc.tile_pool(name="small", bufs=3) as spool, \
             tc.tile_pool(name="big", bufs=3) as bpool, \
             tc.tile_pool(name="ps_ctx", bufs=1, space="PSUM") as pss, \
             tc.tile_pool(name="ps_acc", bufs=2, space="PSUM") as psa, \
             tc.tile_pool(name="ps_big", bufs=2, space="PSUM") as psb:

            piota_t = cpool.tile([P, 1], f32)
            w_t = cpool.tile([P, P], bf16)
            nc.sync.dma_start(piota_t[:], piota_d[:])
            nc.sync.dma_start(w_t[:], W_d[:])

            for k in range(nch):
                # ---- loads ----
                f_t = fpool.tile([P, tiles, P], bf16, tag="f")
                nc.sync.dma_start(f_t[:], F_d[k])

                meta_t = mpool.tile([P, META_W], bf16, tag="meta")
                nc.sync.dma_start(meta_t[:], meta_d[k])
                oh_t = meta_t[:, OH_OFF:OH_OFF + OH_LEN].rearrange(
                    "p (t s) -> p t s", s=PAIRW)
                m01_t = meta_t[:, M01_OFF:M01_OFF + M01_LEN].rearrange(
                    "p (a w) -> p a w", w=WINW)
                recip_t = meta_t[:, RC_OFF:RC_OFF + 1]

                relb_t = rpool.tile([P, swid], bf16, tag="relp")
                nc.sync.dma_start(relb_t[:],
                                  relp4_d[k].broadcast_to([P, swid]))
                ohpT_t = rpool.tile([P, swid], bf16, tag="ohpT")
                nc.vector.tensor_scalar(out=ohpT_t[:], in0=relb_t[:],
                                        scalar1=piota_t[:], scalar2=None,
                                        op0=AluOp.is_equal)

                def slot_base(t):
                    u = pair_of[t]
                    return (u % 4) * PAIRW, u // 4

                # ---- step1: segment sums into slots ----
                order = [1, 0] + list(range(2, tiles))
                region_of = {t: pair_of[t] for t in order}
                first_of_region = {}
                last_of_region = {}
                for t in order:
                    u = region_of[t]
                    first_of_region.setdefault(u, t)
                    last_of_region[u] = t
                ps_s = psa.tile([P, n_pt, P], f32, tag="ps_s",
                                name=f"ps_s_{k}", bufs=1)
                for t in order:
                    base, pt = slot_base(t)
                    u = region_of[t]
                    nc.tensor.matmul(
                        ps_s[base:base + PAIRW, pt, :],
                        oh_t[:, t, :], f_t[:, t, :],
                        start=(first_of_region[u] == t),
                        stop=(last_of_region[u] == t),
                        tile_position=(0, base))

                # ---- merge to sumT [D, w] ----
                ps_sumT = pss.tile([P, WINW], f32, tag="sumT", bufs=1)
                s_sb = spool.tile([P, n_pt, P], bf16, tag="s_sb")
                nc.scalar.copy(s_sb[:], ps_s[:])
                for pt in range(n_pt):
                    nc.tensor.matmul(ps_sumT[:], s_sb[:, pt, :], m01_t[:, pt, :],
                                     start=(pt == 0), stop=(pt == n_pt - 1))

                # ---- ctx: pre = sum_w @ W ; ctxn = tanh(recip * pre) ----
                sumT_sb = spool.tile([P, WINW], bf16, tag="sumT_sb")
                nc.scalar.copy(sumT_sb[:], ps_sumT[:])
                ps_pre = pss.tile([WINW, P], f32, tag="pre", bufs=1)
                nc.tensor.matmul(ps_pre[:], sumT_sb[:], w_t[:],
                                 start=True, stop=True)
                recipf_t = spool.tile([P, 1], f32, tag="recipf")
                nc.vector.tensor_copy(recipf_t[:], recip_t)
                ctxn_t = spool.tile([WINW, P], bf16, tag="ctxn")
                nc.scalar.activation(ctxn_t[:], ps_pre[:], Act.Tanh,
                                     scale=recipf_t[:])

                # ---- 3a: CtxG per tile (K=128 window matmuls) + evac ----
                cg_sb = bpool.tile([P, tiles, P], bf16, tag="cg")
                GRP = 8
                n_g8 = (tiles + GRP - 1) // GRP
                for g in range(n_g8):
                    t0g = g * GRP
                    ng = min(GRP, tiles - t0g)
                    ps_cg = psb.tile([P, GRP * P], f32, tag="bigps",
                                     name=f"ps_cg_{k}_{g}")
                    for i in range(ng):
                        t = t0g + i
                        nc.tensor.matmul(
                            ps_cg[:, i * P:(i + 1) * P],
                            ohpT_t[:, t * P:(t + 1) * P],
                            ctxn_t[:],
                            start=(i % 4 == 0),
                            stop=(i % 4 == 3 or i == ng - 1))
                    srcv = ps_cg[:, :ng * P].rearrange("p (t d) -> p t d", d=P)
                    nc.scalar.copy(cg_sb[:, t0g:t0g + ng, :], srcv)

                # ---- scores = rowsum(f * cg): 2x mult + fold tree ----
                prod_sb = bpool.tile([P, tiles, P], bf16, tag="prod")
                nc.vector.tensor_tensor(out=prod_sb[:], in0=f_t[:],
                                        in1=cg_sb[:], op=AluOp.mult)
                fold1 = spool.tile([P, tiles, P // 2], bf16, tag="fold1")
                nc.vector.tensor_tensor(out=fold1[:],
                                        in0=prod_sb[:, :, :P // 2],
                                        in1=prod_sb[:, :, P // 2:],
                                        op=AluOp.add)
                fold2 = spool.tile([P, tiles, P // 4], bf16, tag="fold2")
                nc.vector.tensor_tensor(out=fold2[:],
                                        in0=fold1[:, :, :P // 4],
                                        in1=fold1[:, :, P // 4:],
                                        op=AluOp.add)
                fold3 = spool.tile([P, tiles, P // 8], bf16, tag="fold3")
                nc.vector.tensor_tensor(out=fold3[:],
                                        in0=fold2[:, :, :P // 8],
                                        in1=fold2[:, :, P // 8:],
                                        op=AluOp.add)
                scores_t = spool.tile([P, tiles], f32, tag="scores")
                nc.vector.tensor_reduce(out=scores_t[:], in_=fold3[:],
                                        axis=mybir.AxisListType.X,
                                        op=AluOp.add)
                scores_b = spool.tile([P, tiles], bf16, tag="scores_b")
                nc.vector.tensor_copy(scores_b[:], scores_t[:])

                # ---- 3b: rep sums ----
                ohsc_t = spool.tile([P, tiles, PAIRW], bf16, tag="ohsc")
                eng = nc.gpsimd if OHSC_ON_GPSIMD else nc.vector
                eng.tensor_tensor(
                    out=ohsc_t[:], in0=oh_t[:],
                    in1=scores_b[:].unsqueeze(2).broadcast_to(
                        [P, tiles, PAIRW]),
                    op=AluOp.mult)
                ps_r = psa.tile([P, n_pt, P], f32, tag="ps_r",
                                name=f"ps_r_{k}", bufs=1)
                for t in order:
                    base, pt = slot_base(t)
                    u = region_of[t]
                    nc.tensor.matmul(
                        ps_r[base:base + PAIRW, pt, :],
                        ohsc_t[:, t, :], f_t[:, t, :],
                        start=(first_of_region[u] == t),
                        stop=(last_of_region[u] == t),
                        tile_position=(0, base))
                r_sb = spool.tile([P, n_pt, P], bf16, tag="r_sb")
                nc.scalar.copy(r_sb[:], ps_r[:])
                nc.sync.dma_start(rep_d[k], r_sb[:])

    nc.compile()
    return nc


def host_prep(features, segment_ids, num_segments, weight_matrix, tpc=TPC,
              strict=True):
    """Numpy preprocessing. Returns (nch, in_maps, meta, cnt) or None if the
    geometry (window spans) doesn't fit for this tpc."""
    N, D = features.shape
    G = int(num_segments)
    seg = np.asarray(segment_ids).astype(np.int64)
    feats = np.asarray(features, dtype=np.float32)
    W = np.asarray(weight_matrix, dtype=np.float32)

    chunk = tpc * P
    tiles = tpc + 1
    nodes = tiles * P
    n_pairs = tpc // 2
    n_pt = (n_pairs + 3) // 4

    bnd = np.searchsorted(seg, np.arange(G + 1))
    cnt = np.diff(bnd)
    if cnt.max() > P:
        assert not strict, f"segment with {cnt.max()} nodes > {P}"
        return None
    recip_full = np.where(cnt > 0, 1.0 / np.maximum(cnt, 1), 0.0).astype(np.float32)

    cuts = [0]
    for c in range(1, NCORES):
        gidx = min(int(np.searchsorted(bnd, round(c * N / NCORES))), G)
        cuts.append(int(bnd[gidx]))
    cuts.append(N)
    counts = [cuts[c + 1] - cuts[c] for c in range(NCORES)]
    nch = max(1, math.ceil(max(counts) / chunk))

    in_maps = []
    meta = []
    for c in range(NCORES):
        n0, n1 = cuts[c], cuts[c + 1]
        Nc = n1 - n0
        segl = seg[n0:n1]

        f_pad = np.zeros((P + nch * chunk, D), BF16)
        f_pad[P:P + Nc] = feats[n0:n1].astype(BF16)
        f_in = np.lib.stride_tricks.sliding_window_view(
            f_pad, (nodes, D))[::chunk, 0][:nch]
        f_in = np.ascontiguousarray(
            f_in.reshape(nch, tiles, P, D).transpose(0, 2, 1, 3))

        v = np.arange(Nc)
        chunk_of = v // chunk
        g_lo, g_hi = int(segl[0]), int(segl[-1]) + 1
        own = (bnd[np.arange(g_lo, g_hi) + 1] - 1 - n0) // chunk
        own_of_node = own[segl - g_lo]
        valid = own_of_node == chunk_of

        pw = np.full((nch, n_pairs), 0, np.int64)
        for k in range(nch):
            for u in range(n_pairs):
                i = k * chunk + u * 2 * P
                pw[k, u] = segl[min(i, Nc - 1)]
        wk = pw[:, 0]

        relp = np.where(valid, segl - pw[chunk_of, ((v % chunk) // P) // 2],
                        MASK).astype(np.float32)
        relw = np.where(valid, segl - wk[chunk_of], MASK).astype(np.float32)

        rel32 = np.full((nch, P, tiles), MASK, np.float32)
        brow = np.full((nch, tiles * P), MASK, np.float32)
        pad = np.full(nch * chunk - Nc, MASK, np.float32)
        rp = np.concatenate([relp, pad]).reshape(nch, tpc, P)
        rw = np.concatenate([relw, pad]).reshape(nch, tpc, P)
        rel32[:, :, 1:] = rp.transpose(0, 2, 1)
        brow[:, P:] = rw.reshape(nch, -1)

        for k in range(1, nch):
            lo = k * chunk - P
            if lo >= Nc:
                continue
            hi = min(k * chunk, Nc)
            idx = np.arange(lo, hi)
            bvalid = own_of_node[idx] == k
            br = np.where(bvalid, segl[idx] - wk[k], MASK).astype(np.float32)
            rel32[k, :hi - lo, 0] = br
            brow[k, :hi - lo] = br

        # geometry checks (fall back to smaller tpc on overflow)
        rel_ok = rel32[rel32 > MASK / 2]
        brow_ok = brow[brow > MASK / 2]
        bad = (rel_ok.size and (rel_ok.min() < 0 or rel_ok.max() >= PAIRW)) or \
              (brow_ok.size and (brow_ok.min() < 0 or brow_ok.max() >= WINW)) or \
              (pw - wk[:, None]).max() + PAIRW > WINW
        if bad:
            assert not strict, "window overflow"
            return None

        oh01 = (rel32[..., None] ==
                np.arange(PAIRW, dtype=np.float32)).astype(BF16)

        # relp4 [nch, 1, tiles*P]: window-relative ids, node order
        relp4 = brow.reshape(nch, 1, tiles * P).astype(np.float32)

        sl = np.arange(P)
        m01 = np.zeros((nch, P, n_pt, WINW), np.float32)
        for pt in range(n_pt):
            u = np.minimum(4 * pt + sl // PAIRW, n_pairs - 1)
            t0r = pw[:, u] - wk[:, None]                  # [nch, P]
            m01[:, :, pt, :] = (np.arange(WINW)[None, None, :]
                                == (t0r + (sl % PAIRW))[:, :, None])

        gw = wk[:, None] + np.arange(WINW)[None, :]
        recip = np.where(gw < G, recip_full[np.minimum(gw, G - 1)], 0.0
                         ).astype(np.float32)
        # recip is consumed as a per-partition activation scale [P, 1]:
        # partition p holds recip for window row p
        # (ctxn rows = window rows)

        meta_pack = np.concatenate([
            oh01.reshape(nch, P, tiles * PAIRW),
            m01.reshape(nch, P, n_pt * WINW),
            recip.reshape(nch, P, 1),
        ], axis=2).astype(BF16)

        in_maps.append({
            "f_in": f_in,
            "meta": meta_pack,
            "relp4": relp4.astype(BF16),
            "w_in": W.astype(BF16),
            "piota": np.arange(P, dtype=np.float32)[:, None],
        })
        meta.append({"n0": n0, "n1": n1, "g_lo": g_lo, "g_hi": g_hi,
                     "own": own, "wk": wk, "pw": pw, "tpc": tpc})
    return nch, in_maps, meta, cnt


def assemble(results, meta, G, D, cnt=None):
    rep = np.zeros((G, D), np.float32)
    for c in range(NCORES):
        out = np.asarray(results[c]["rep_out"], dtype=np.float32)
        m = meta[c]
        tpc = m["tpc"]
        n_pt = (tpc // 2 + 3) // 4
        pw = m["pw"]
        nch = pw.shape[0]
        s = np.arange(P)
        u = np.minimum((s // PAIRW)[None, :] + 4 * np.arange(n_pt)[:, None],
                       tpc // 2 - 1)
        tgt = pw[:, u] + (s % PAIRW)[None, None, :]      # [nch, n_pt, P]
        part = out.transpose(0, 2, 1, 3).reshape(nch * n_pt * P, D)
        tgt = tgt.transpose(0, 1, 2).reshape(-1)
        ok = tgt < G
        np.add.at(rep, tgt[ok], part[ok])
    return rep


_LAST_RUN = {}


def kernel(features, segment_ids, num_segments, weight_matrix):
    from concourse.bass_utils import run_bass_kernel_spmd
    _ensure_ntff_hook()

    G = int(num_segments)
    D = features.shape[1]
    prep = host_prep(features, segment_ids, num_segments, weight_matrix,
                     tpc=32, strict=False)
    tpc = 32
    if prep is None:
        tpc = 16
        prep = host_prep(features, segment_ids, num_segments, weight_matrix,
                         tpc=16, strict=True)
    nch, in_maps, meta, cnt = prep
    nc = build_program(nch, tpc)
    trace = bool(int(os.environ.get("BASS_KERNEL_TRACE", "0")))
    kw = {}
    if trace:
        kw["trace"] = True
        kw["tmpdir"] = os.environ.get("BASS_KERNEL_TRACE_DIR") or None
    res = run_bass_kernel_spmd(nc, in_maps, core_ids=list(range(NCORES)), **kw)
    _LAST_RUN["exec_time_ns"] = res.exec_time_ns
    _LAST_RUN["res"] = res
    return assemble(res.results, meta, G, D, cnt)


# revision 14
# speedup vs baseline: 1.0175x; 1.0175x over previous
"""Trainium2 Bass kernel for nn_AttentionModule (segment_reduce).

Computation (per reference):
    wx   = features @ W
    s_g  = segment_sum(wx);  cnt_g = segment counts
    ctx  = tanh(s_g / max(cnt,1))            [G, D]
    score_n = <f_n, ctx[seg_n]>
    rep_g = segment_sum(score_n * f_n)       [G, D]

Key identity: segment_sum(F @ W) = segment_sum(F) @ W, so pass A is a pure
segment-sum.  Device algorithm (SPMD over 8 cores, segment-aligned shard
per core, one uniform compiled program; all data-dependence flows through
input tensors, never instruction addresses):

  per 4096-node chunk (32 new 128-node tiles + 1 boundary re-read tile):
    - pair one-hot oh01 [node, 32] host-built; segment-sum via PE matmuls
      into 32-aligned psum slot ranges; slot->window merge matmul (m01)
    - ctx = tanh((sum @ W) * recip)  [window 128, D]
    - unmerge matmul (m01T) redistributes ctx window rows back to pair
      slots: ctxp [slot 128, D]
    - transposed pair one-hot ohpT [strip*32+s, node] built on DVE via a
      single is_equal against a p%32 iota from a strip-packed broadcast
      relp4 (only 0.3MB/chunk vs 1.1MB for a full window one-hot)
    - ctx gather per tile: K=32 row-strip matmuls ohpT.T @ ctxp -> CtxG
      [node, D] in PSUM, evacuated to SBUF bf16 by ScalarE
    - scores = rowsum(F * CtxG): DVE 2x-mode bf16 multiply + single fused
      reduce;  rep via scores-scaled one-hot matmul (GpSimd does the
      scale), written chunk-major bf16 to DRAM
  host: numpy pre/post-processing (index metadata, shard assembly).
"""

import os
import sys
import math
from functools import lru_cache

for _p in ("/opt/trn_rl_repo", "/root/.axon_site/_ro/trn_rl_repo"):
    if os.path.isdir(_p) and _p not in sys.path:
        sys.path.insert(0, _p)

import numpy as np
import ml_dtypes

BF16 = ml_dtypes.bfloat16

P = 128          # partitions
TPC = 32         # new 128-node tiles per chunk
TILES = TPC + 1  # + boundary tile (tile 0)
WINW = 128       # chunk segment-window width
PAIRW = 32       # pair segment-window width
NCORES = 8
MASK = -1000.0

OHSC_ON_GPSIMD = bool(int(os.environ.get("K_OHSC_GPSIMD", "1")))
EVAC_DVE_GROUPS = int(os.environ.get("K_EVAC_DVE", "0"))  # of the 9 cg groups


def _strip_tiles(tpc):
    """tiles assigned to each of the 4 row strips, in block order."""
    strips = []
    for r in range(4):
        ts = []
        if r == 0:
            ts.append(0)
        for u in range(r, tpc // 2, 4):
            ts.extend([2 * u + 1, 2 * u + 2])
        strips.append(ts)
    return strips


def _ensure_ntff_hook():
    """Register the axon NTFF profile hook if the boot couldn't (antenv stub)."""
    import types
    try:
        import antenv  # noqa
    except ImportError:
        return
    if "antenv.axon_hooks" in sys.modules:
        return
    hooks = types.ModuleType("antenv.axon_hooks")
    holder = [None]
    hooks.set_axon_ntff_profile_hook = lambda h: holder.__setitem__(0, h)
    hooks.get_axon_ntff_profile_hook = lambda: holder[0]
    sys.modules["antenv.axon_hooks"] = hooks
    import antenv
    antenv.axon_hooks = hooks
    try:
        from trn_agent_boot.trn_boot import _ntff_profile_via_ctypes
        so = "/opt/axon/libaxon_pjrt.so"
        if os.path.exists(so):
            hooks.set_axon_ntff_profile_hook(_ntff_profile_via_ctypes(so))
    except Exception:
        pass


@lru_cache(maxsize=4)
def build_program(nch: int, tpc: int = TPC):
    """Build + compile the uniform per-core Bass program."""
    import concourse.bass as bass  # noqa
    import concourse.mybir as mybir
    from concourse import bacc, tile

    tiles = tpc + 1
    n_pairs = tpc // 2
    n_pt = (n_pairs + 3) // 4          # psum partial tiles (4 pairs each)
    n_g4 = (tiles + 3) // 4            # ctx-gather groups of 4 tiles
    swid = tiles * P                    # window one-hot width
    f32 = mybir.dt.float32
    bf16 = mybir.dt.bfloat16

    # byte offsets inside the packed meta tensor (bf16 elements)
    OH_OFF = 0
    OH_LEN = tiles * PAIRW
    M01_OFF = OH_OFF + OH_LEN
    M01_LEN = n_pt * WINW
    RC_OFF = M01_OFF + M01_LEN
    META_W = RC_OFF + 1

    nc = bacc.Bacc("TRN2", target_bir_lowering=False, debug=False,
                   num_devices=NCORES)

    # ---- DRAM I/O ----
    F_d = nc.dram_tensor("f_in", [nch, P, tiles, P], bf16,
                         kind="ExternalInput").ap()
    meta_d = nc.dram_tensor("meta", [nch, P, META_W], bf16,
                            kind="ExternalInput").ap()
    relp4_d = nc.dram_tensor("relp4", [nch, 1, swid], bf16,
                             kind="ExternalInput").ap()
    piota_d = nc.dram_tensor("piota", [P, 1], f32,
                             kind="ExternalInput").ap()
    W_d = nc.dram_tensor("w_in", [P, P], bf16, kind="ExternalInput").ap()
    rep_d = nc.dram_tensor("rep_out", [nch, P, n_pt, P], bf16,
                           kind="ExternalOutput").ap()

    AluOp = mybir.AluOpType
    Act = mybir.ActivationFunctionType

    pair_of = {t: ((t - 1) // 2 if t >= 1 else 0) for t in range(tiles)}

    with tile.TileContext(nc) as tc:
        with tc.tile_pool(name="const", bufs=1) as cpool, \
             tc.tile_pool(name="fpool", bufs=4) as fpool, \
             tc.tile_pool(name="mpool", bufs=4) as mpool, \
             tc.tile_pool(name="rpool", bufs=4) as rpool, \
             tc.tile_pool(name="small", bufs=3) as spool, \
             tc.tile_pool(name="big", bufs=3) as bpool, \
             tc.tile_pool(name="ps_ctx", bufs=1, space="PSUM") as pss, \
             tc.tile_pool(name="ps_acc", bufs=2, space="PSUM") as psa, \
             tc.tile_pool(name="ps_big", bufs=2, space="PSUM") as psb:

            piota_t = cpool.tile([P, 1], f32)
            w_t = cpool.tile([P, P], bf16)
            nc.sync.dma_start(piota_t[:], piota_d[:])
            nc.sync.dma_start(w_t[:], W_d[:])

            for k in range(nch):
                # ---- loads ----
                f_t = fpool.tile([P, tiles, P], bf16, tag="f")
                nc.sync.dma_start(f_t[:], F_d[k])

                meta_t = mpool.tile([P, META_W], bf16, tag="meta")
                nc.sync.dma_start(meta_t[:], meta_d[k])
                oh_t = meta_t[:, OH_OFF:OH_OFF + OH_LEN].rearrange(
                    "p (t s) -> p t s", s=PAIRW)
                m01_t = meta_t[:, M01_OFF:M01_OFF + M01_LEN].rearrange(
                    "p (a w) -> p a w", w=WINW)
                recip_t = meta_t[:, RC_OFF:RC_OFF + 1]

                relb_t = rpool.tile([P, swid], bf16, tag="relp")
                nc.sync.dma_start(relb_t[:],
                                  relp4_d[k].broadcast_to([P, swid]))
                ohpT_t = rpool.tile([P, swid], bf16, tag="ohpT")
                nc.vector.tensor_scalar(out=ohpT_t[:], in0=relb_t[:],
                                        scalar1=piota_t[:], scalar2=None,
                                        op0=AluOp.is_equal)

                def slot_base(t):
                    u = pair_of[t]
                    return (u % 4) * PAIRW, u // 4

                # ---- step1: segment sums into slots ----
                order = [1, 0] + list(range(2, tiles))
                region_of = {t: pair_of[t] for t in order}
                first_of_region = {}
                last_of_region = {}
                for t in order:
                    u = region_of[t]
                    first_of_region.setdefault(u, t)
                    last_of_region[u] = t
                ps_s = psa.tile([P, n_pt, P], f32, tag="ps_s",
                                name=f"ps_s_{k}", bufs=1)
                for t in order:
                    base, pt = slot_base(t)
                    u = region_of[t]
                    nc.tensor.matmul(
                        ps_s[base:base + PAIRW, pt, :],
                        oh_t[:, t, :], f_t[:, t, :],
                        start=(first_of_region[u] == t),
                        stop=(last_of_region[u] == t),
                        tile_position=(0, base))

                # ---- merge to sumT [D, w] ----
                ps_sumT = pss.tile([P, WINW], f32, tag="sumT", bufs=1)
                s_sb = spool.tile([P, n_pt, P], bf16, tag="s_sb")
                nc.scalar.copy(s_sb[:], ps_s[:])
                for pt in range(n_pt):
                    nc.tensor.matmul(ps_sumT[:], s_sb[:, pt, :], m01_t[:, pt, :],
                                     start=(pt == 0), stop=(pt == n_pt - 1))

                # ---- ctx: pre = sum_w @ W ; ctxn = tanh(recip * pre) ----
                sumT_sb = spool.tile([P, WINW], bf16, tag="sumT_sb")
                nc.scalar.copy(sumT_sb[:], ps_sumT[:])
                ps_pre = pss.tile([WINW, P], f32, tag="pre", bufs=1)
                nc.tensor.matmul(ps_pre[:], sumT_sb[:], w_t[:],
                                 start=True, stop=True)
                recipf_t = spool.tile([P, 1], f32, tag="recipf")
                nc.vector.tensor_copy(recipf_t[:], recip_t)
                ctxn_t = spool.tile([WINW, P], bf16, tag="ctxn")
                nc.scalar.activation(ctxn_t[:], ps_pre[:], Act.Tanh,
                                     scale=recipf_t[:])

                # ---- 3a: CtxG per tile (K=128 window matmuls) ----
                # group 1 is multiplied straight out of PSUM by DVE; the
                # rest are evacuated bf16 by ScalarE then multiplied at 2x
                cg_sb = bpool.tile([P, tiles, P], bf16, tag="cg")
                prod_sb = bpool.tile([P, tiles, P], bf16, tag="prod")
                GRP = 8
                DVE_G = 1
                n_g8 = (tiles + GRP - 1) // GRP
                for g in range(n_g8):
                    t0g = g * GRP
                    ng = min(GRP, tiles - t0g)
                    ps_cg = psb.tile([P, GRP * P], f32, tag="bigps",
                                     name=f"ps_cg_{k}_{g}")
                    for i in range(ng):
                        t = t0g + i
                        nc.tensor.matmul(
                            ps_cg[:, i * P:(i + 1) * P],
                            ohpT_t[:, t * P:(t + 1) * P],
                            ctxn_t[:],
                            start=(i % 4 == 0),
                            stop=(i % 4 == 3 or i == ng - 1))
                    srcv = ps_cg[:, :ng * P].rearrange("p (t d) -> p t d", d=P)
                    if g == DVE_G:
                        nc.vector.tensor_tensor(
                            out=prod_sb[:, t0g:t0g + ng, :],
                            in0=f_t[:, t0g:t0g + ng, :],
                            in1=srcv, op=AluOp.mult)
                    else:
                        nc.scalar.copy(cg_sb[:, t0g:t0g + ng, :], srcv)
                        nc.vector.tensor_tensor(
                            out=prod_sb[:, t0g:t0g + ng, :],
                            in0=f_t[:, t0g:t0g + ng, :],
                            in1=cg_sb[:, t0g:t0g + ng, :], op=AluOp.mult)
                fold1 = spool.tile([P, tiles, P // 2], bf16, tag="fold1")
                nc.vector.tensor_tensor(out=fold1[:],
                                        in0=prod_sb[:, :, :P // 2],
                                        in1=prod_sb[:, :, P // 2:],
                                        op=AluOp.add)
                fold2 = spool.tile([P, tiles, P // 4], bf16, tag="fold2")
                nc.vector.tensor_tensor(out=fold2[:],
                                        in0=fold1[:, :, :P // 4],
                                        in1=fold1[:, :, P // 4:],
                                        op=AluOp.add)
                fold3 = spool.tile([P, tiles, P // 8], bf16, tag="fold3")
                nc.vector.tensor_tensor(out=fold3[:],
                                        in0=fold2[:, :, :P // 8],
                                        in1=fold2[:, :, P // 8:],
                                        op=AluOp.add)
                scores_t = spool.tile([P, tiles], f32, tag="scores")
                nc.vector.tensor_reduce(out=scores_t[:], in_=fold3[:],
                                        axis=mybir.AxisListType.X,
                                        op=AluOp.add)
                scores_b = spool.tile([P, tiles], bf16, tag="scores_b")
                nc.vector.tensor_copy(scores_b[:], scores_t[:])

                # ---- 3b: rep sums ----
                ohsc_t = spool.tile([P, tiles, PAIRW], bf16, tag="ohsc")
                eng = nc.gpsimd if OHSC_ON_GPSIMD else nc.vector
                eng.tensor_tensor(
                    out=ohsc_t[:], in0=oh_t[:],
                    in1=scores_b[:].unsqueeze(2).broadcast_to(
                        [P, tiles, PAIRW]),
                    op=AluOp.mult)
                ps_r = psa.tile([P, n_pt, P], f32, tag="ps_r",
                                name=f"ps_r_{k}", bufs=1)
                for t in order:
                    base, pt = slot_base(t)
                    u = region_of[t]
                    nc.tensor.matmul(
                        ps_r[base:base + PAIRW, pt, :],
                        ohsc_t[:, t, :], f_t[:, t, :],
                        start=(first_of_region[u] == t),
                        stop=(last_of_region[u] == t),
                        tile_position=(0, base))
                r_sb = spool.tile([P, n_pt, P], bf16, tag="r_sb")
                nc.scalar.copy(r_sb[:], ps_r[:])
                nc.sync.dma_start(rep_d[k], r_sb[:])

    nc.compile()
    return nc


def host_prep(features, segment_ids, num_segments, weight_matrix, tpc=TPC,
              strict=True):
    """Numpy preprocessing. Returns (nch, in_maps, meta, cnt) or None if the
    geometry (window spans) doesn't fit for this tpc."""
    N, D = features.shape
    G = int(num_segments)
    seg = np.asarray(segment_ids).astype(np.int64)
    feats = np.asarray(features, dtype=np.float32)
    W = np.asarray(weight_matrix, dtype=np.float32)

    chunk = tpc * P
    tiles = tpc + 1
    nodes = tiles * P
    n_pairs = tpc // 2
    n_pt = (n_pairs + 3) // 4

    bnd = np.searchsorted(seg, np.arange(G + 1))
    cnt = np.diff(bnd)
    if cnt.max() > P:
        assert not strict, f"segment with {cnt.max()} nodes > {P}"
        return None
    recip_full = np.where(cnt > 0, 1.0 / np.maximum(cnt, 1), 0.0).astype(np.float32)

    cuts = [0]
    for c in range(1, NCORES):
        gidx = min(int(np.searchsorted(bnd, round(c * N / NCORES))), G)
        cuts.append(int(bnd[gidx]))
    cuts.append(N)
    counts = [cuts[c + 1] - cuts[c] for c in range(NCORES)]
    nch = max(1, math.ceil(max(counts) / chunk))

    in_maps = []
    meta = []
    for c in range(NCORES):
        n0, n1 = cuts[c], cuts[c + 1]
        Nc = n1 - n0
        segl = seg[n0:n1]

        f_pad = np.zeros((P + nch * chunk, D), BF16)
        f_pad[P:P + Nc] = feats[n0:n1].astype(BF16)
        f_in = np.lib.stride_tricks.sliding_window_view(
            f_pad, (nodes, D))[::chunk, 0][:nch]
        f_in = np.ascontiguousarray(
            f_in.reshape(nch, tiles, P, D).transpose(0, 2, 1, 3))

        v = np.arange(Nc)
        chunk_of = v // chunk
        g_lo, g_hi = int(segl[0]), int(segl[-1]) + 1
        own = (bnd[np.arange(g_lo, g_hi) + 1] - 1 - n0) // chunk
        own_of_node = own[segl - g_lo]
        valid = own_of_node == chunk_of

        pw = np.full((nch, n_pairs), 0, np.int64)
        for k in range(nch):
            for u in range(n_pairs):
                i = k * chunk + u * 2 * P
                pw[k, u] = segl[min(i, Nc - 1)]
        wk = pw[:, 0]

        relp = np.where(valid, segl - pw[chunk_of, ((v % chunk) // P) // 2],
                        MASK).astype(np.float32)
        relw = np.where(valid, segl - wk[chunk_of], MASK).astype(np.float32)

        rel32 = np.full((nch, P, tiles), MASK, np.float32)
        brow = np.full((nch, tiles * P), MASK, np.float32)
        pad = np.full(nch * chunk - Nc, MASK, np.float32)
        rp = np.concatenate([relp, pad]).reshape(nch, tpc, P)
        rw = np.concatenate([relw, pad]).reshape(nch, tpc, P)
        rel32[:, :, 1:] = rp.transpose(0, 2, 1)
        brow[:, P:] = rw.reshape(nch, -1)

        for k in range(1, nch):
            lo = k * chunk - P
            if lo >= Nc:
                continue
            hi = min(k * chunk, Nc)
            idx = np.arange(lo, hi)
            bvalid = own_of_node[idx] == k
            br = np.where(bvalid, segl[idx] - wk[k], MASK).astype(np.float32)
            rel32[k, :hi - lo, 0] = br
            brow[k, :hi - lo] = br

        # geometry checks (fall back to smaller tpc on overflow)
        rel_ok = rel32[rel32 > MASK / 2]
        brow_ok = brow[brow > MASK / 2]
        bad = (rel_ok.size and (rel_ok.min() < 0 or rel_ok.max() >= PAIRW)) or \
              (brow_ok.size and (brow_ok.min() < 0 or brow_ok.max() >= WINW)) or \
              (pw - wk[:, None]).max() + PAIRW > WINW
        if bad:
            assert not strict, "window overflow"
            return None

        oh01 = (rel32[..., None] ==
                np.arange(PAIRW, dtype=np.float32)).astype(BF16)

        # relp4 [nch, 1, tiles*P]: window-relative ids, node order
        relp4 = brow.reshape(nch, 1, tiles * P).astype(np.float32)

        sl = np.arange(P)
        m01 = np.zeros((nch, P, n_pt, WINW), np.float32)
        for pt in range(n_pt):
            u = np.minimum(4 * pt + sl // PAIRW, n_pairs - 1)
            t0r = pw[:, u] - wk[:, None]                  # [nch, P]
            m01[:, :, pt, :] = (np.arange(WINW)[None, None, :]
                                == (t0r + (sl % PAIRW))[:, :, None])

        gw = wk[:, None] + np.arange(WINW)[None, :]
        recip = np.where(gw < G, recip_full[np.minimum(gw, G - 1)], 0.0
                         ).astype(np.float32)
        # recip is consumed as a per-partition activation scale [P, 1]:
        # partition p holds recip for window row p
        # (ctxn rows = window rows)

        meta_pack = np.concatenate([
            oh01.reshape(nch, P, tiles * PAIRW),
            m01.reshape(nch, P, n_pt * WINW),
            recip.reshape(nch, P, 1),
        ], axis=2).astype(BF16)

        in_maps.append({
            "f_in": f_in,
            "meta": meta_pack,
            "relp4": relp4.astype(BF16),
            "w_in": W.astype(BF16),
            "piota": np.arange(P, dtype=np.float32)[:, None],
        })
        meta.append({"n0": n0, "n1": n1, "g_lo": g_lo, "g_hi": g_hi,
                     "own": own, "wk": wk, "pw": pw, "tpc": tpc})
    return nch, in_maps, meta, cnt


def assemble(results, meta, G, D, cnt=None):
    rep = np.zeros((G, D), np.float32)
    for c in range(NCORES):
        out = np.asarray(results[c]["rep_out"], dtype=np.float32)
        m = meta[c]
        tpc = m["tpc"]
        n_pt = (tpc // 2 + 3) // 4
        pw = m["pw"]
        nch = pw.shape[0]
        s = np.arange(P)
        u = np.minimum((s // PAIRW)[None, :] + 4 * np.arange(n_pt)[:, None],
                       tpc // 2 - 1)
        tgt = pw[:, u] + (s % PAIRW)[None, None, :]      # [nch, n_pt, P]
        part = out.transpose(0, 2, 1, 3).reshape(nch * n_pt * P, D)
        tgt = tgt.transpose(0, 1, 2).reshape(-1)
        ok = tgt < G
        np.add.at(rep, tgt[ok], part[ok])
    return rep


_LAST_RUN = {}


def kernel(features, segment_ids, num_segments, weight_matrix):
    from concourse.bass_utils import run_bass_kernel_spmd
    _ensure_ntff_hook()

    G = int(num_segments)
    D = features.shape[1]
    prep = host_prep(features, segment_ids, num_segments, weight_matrix,
                     tpc=32, strict=False)
    tpc = 32
    if prep is None:
        tpc = 16
        prep = host_prep(features, segment_ids, num_segments, weight_matrix,
                         tpc=16, strict=True)
    nch, in_maps, meta, cnt = prep
    nc = build_program(nch, tpc)
    trace = bool(int(os.environ.get("BASS_KERNEL_TRACE", "0")))
    kw = {}
    if trace:
        kw["trace"] = True
        kw["tmpdir"] = os.environ.get("BASS_KERNEL_TRACE_DIR") or None
    res = run_bass_kernel_spmd(nc, in_maps, core_ids=list(range(NCORES)), **kw)
    _LAST_RUN["exec_time_ns"] = res.exec_time_ns
    _LAST_RUN["res"] = res
    return assemble(res.results, meta, G, D, cnt)
